# revision 7
# baseline (speedup 1.0000x reference)
"""Trainium2 kernel for nn_ChunkAggregator (per-block vocab histogram).

Reference semantics (B=8, L=16384, H=2, BLOCK=128, VOCAB=32000):
  blocks   = tokens.reshape(B, 128, 128, 2)
  cat_ids  = blocks[:, :, 0, :].reshape(B, 256)
  hist     = per-(batch, block) histogram of the 256 token ids over 32000 bins
  new_tokens = concat([cat_ids, tokens.reshape(B, -1)], axis=1)

Strategy: batch-parallel over 8 NeuronCores (1 batch per core). On each core
the histogram for block b is computed with a factorized one-hot matmul:
  id = hi*256 + lo, hi in [0,125), lo in [0,256)
  hist2d[hi, lo] = sum_k onehot125(hi_k) * onehot256(lo_k)  (PE matmul, K=256)
which yields exact integer counts in PSUM fp32 (bf16 one-hots are exact 0/1,
counts <= 256 are exact). The flattened (hi, lo) axis is exactly the vocab
axis, so PSUM [125, 256] -> SBUF -> strided DMA writes the full 32000-wide
row for each block, zeros included (no scatter needed; duplicate ids are
handled by the matmul accumulation).
"""

import numpy as np
from contextlib import ExitStack

import concourse.bacc as bacc
import concourse.bass as bass
import concourse.mybir as mybir
import concourse.tile as tile
from concourse.bass_utils import run_bass_kernel_spmd

P = 128          # SBUF partitions == positions per block
BLOCK = 128
NB = 128         # blocks per batch (L // BLOCK)
L = NB * BLOCK   # 16384
H = 2
VOCAB = 32000
MHI = 125        # vocab = MHI * NLO
NLO = 256
GROUP = 16       # blocks per output DMA
N_CORES = 8

f32 = mybir.dt.float32
bf16 = mybir.dt.bfloat16
i32 = mybir.dt.int32
i16 = mybir.dt.int16


def build_nc(copy_split=0, iters=1):
    """One core's program: tok [128, 256] int32 -> hist [128, 32000] f32.

    tok layout: tok[pos, 2*b + h] = tokens[b*128 + pos, h] (host pre-arranged
    so the DMA in is fully contiguous).
    copy_split: every copy_split-th PSUM->SBUF copy goes to the scalar engine
    to offload the vector engine (0 = all on vector).
    iters: emit the whole body this many times (benchmarking only).
    """
    nc = bacc.Bacc("TRN2", target_bir_lowering=False, debug=False)
    tok_in = nc.dram_tensor("tok", [P, NB * H], i32, kind="ExternalInput")
    hist_out = nc.dram_tensor("hist", [NB, VOCAB], f32, kind="ExternalOutput")

    with ExitStack() as ctx:
        tc = ctx.enter_context(tile.TileContext(nc))
        const = ctx.enter_context(tc.tile_pool(name="const", bufs=1))
        work = ctx.enter_context(tc.tile_pool(name="work", bufs=1))
        oh_pool = ctx.enter_context(tc.tile_pool(name="onehot", bufs=2))
        psum = ctx.enter_context(tc.tile_pool(name="psum", bufs=8, space="PSUM"))
        out_pool = ctx.enter_context(tc.tile_pool(name="outp", bufs=3))

        # iota row 0..255 replicated on every partition, as bf16
        iota16 = const.tile([P, NLO], i16)
        nc.gpsimd.iota(iota16[:], pattern=[[1, NLO]], base=0, channel_multiplier=0)
        iota_bf = const.tile([P, NLO], bf16)
        nc.vector.tensor_copy(iota_bf[:], iota16[:])

        for _ in range(iters):
            tok = work.tile([P, NB * H], i32, tag="tok")
            nc.sync.dma_start(tok[:], tok_in[:])

            # hi = tok >> 8, lo = tok & 255 (exact), to bf16 (ints < 256: exact)
            hi32 = work.tile([P, NB * H], i32, tag="hi32")
            lo32 = work.tile([P, NB * H], i32, tag="lo32")
            nc.vector.tensor_scalar(
                hi32[:], tok[:], 8, None, mybir.AluOpType.logical_shift_right
            )
            nc.vector.tensor_scalar(
                lo32[:], tok[:], 255, None, mybir.AluOpType.bitwise_and
            )
            hi_bf = work.tile([P, NB * H], bf16, tag="hi_bf")
            lo_bf = work.tile([P, NB * H], bf16, tag="lo_bf")
            nc.vector.tensor_copy(hi_bf[:], hi32[:])
            nc.vector.tensor_copy(lo_bf[:], lo32[:])

            n_groups = NB // GROUP
            C = GROUP * H  # token columns per group
            for g in range(n_groups):
                cs = g * C
                A = oh_pool.tile([P, C, MHI], bf16, tag="A")
                Bt = oh_pool.tile([P, C, NLO], bf16, tag="B")
                nc.vector.tensor_tensor(
                    A[:, :, :],
                    hi_bf[:, cs:cs + C, None].to_broadcast([P, C, MHI]),
                    iota_bf[:, None, :MHI].to_broadcast([P, C, MHI]),
                    mybir.AluOpType.is_equal,
                )
                nc.vector.tensor_tensor(
                    Bt[:, :, :],
                    lo_bf[:, cs:cs + C, None].to_broadcast([P, C, NLO]),
                    iota_bf[:, None, :].to_broadcast([P, C, NLO]),
                    mybir.AluOpType.is_equal,
                )
                og = out_pool.tile([P, GROUP, NLO], f32, tag="og")
                for j in range(GROUP):
                    pt = psum.tile([MHI, NLO], f32, tag="pt")
                    nc.tensor.matmul(
                        pt[:], A[:, 2 * j, :], Bt[:, 2 * j, :],
                        start=True, stop=False,
                    )
                    nc.tensor.matmul(
                        pt[:], A[:, 2 * j + 1, :], Bt[:, 2 * j + 1, :],
                        start=False, stop=True,
                    )
                    if copy_split and (j % copy_split == copy_split - 1):
                        nc.scalar.copy(og[:MHI, j, :], pt[:, :])
                    else:
                        nc.vector.tensor_copy(og[:MHI, j, :], pt[:, :])
                nc.sync.dma_start(
                    hist_out[g * GROUP:(g + 1) * GROUP, :].rearrange(
                        "b (m l) -> m b l", m=MHI
                    ),
                    og[:MHI, :, :],
                )
    nc.compile()
    return nc


_CACHE = {}


def _get_nc():
    if "nc" not in _CACHE:
        _CACHE["nc"] = build_nc()
    return _CACHE["nc"]


def shard_tokens(tokens):
    """Per-core input maps: batch i -> core i, rearranged to [pos, (block, h)]."""
    B_ = tokens.shape[0]
    maps = []
    for i in range(B_):
        tok_sb = np.ascontiguousarray(
            tokens[i]
            .reshape(NB, BLOCK, H)
            .transpose(1, 0, 2)
            .reshape(BLOCK, NB * H)
            .astype(np.int32)
        )
        maps.append({"tok": tok_sb})
    return maps


def kernel(tokens, cat_embed_f, W_num):
    tokens = np.asarray(tokens)
    B_, L_, H_ = tokens.shape
    assert (L_, H_) == (L, H) and B_ == N_CORES

    # Passthrough outputs (pure reshuffles of the input)
    cat_ids = np.ascontiguousarray(
        tokens.reshape(B_, NB, BLOCK, H_)[:, :, 0, :].reshape(B_, NB * H_)
    ).astype(np.int32)
    new_tokens = np.concatenate(
        [cat_ids, tokens.reshape(B_, L_ * H_)], axis=1
    ).astype(np.int32)

    nc = _get_nc()
    res = run_bass_kernel_spmd(nc, shard_tokens(tokens), list(range(N_CORES)))
    hist = np.stack(
        [res.results[i]["hist"] for i in range(B_)], axis=0
    ).astype(np.float32)
    return new_tokens, cat_ids, hist


# revision 21
# speedup vs baseline: 160.8606x; 160.8606x over previous
"""Trainium2 kernel for nn_ChunkAggregator (per-block vocab histogram).

Reference semantics (B=8, L=16384, H=2, BLOCK=128, VOCAB=32000):
  blocks   = tokens.reshape(B, 128, 128, 2)
  cat_ids  = blocks[:, :, 0, :].reshape(B, 256)
  hist     = per-(batch, block) histogram of the 256 token ids over 32000 bins
  new_tokens = concat([cat_ids, tokens.reshape(B, -1)], axis=1)

Strategy: batch-parallel over 8 NeuronCores (1 batch per core). On each core
the histogram for block b is computed with a factorized one-hot matmul:
  id = hi*256 + lo, hi in [0,125), lo in [0,256)
  hist2d[hi, lo] = sum_k onehot125(hi_k) * onehot256(lo_k)  (PE matmul, K=256)
which yields exact integer counts in PSUM fp32 (bf16 one-hots are exact 0/1,
counts <= 256 are exact). The flattened (hi, lo) axis is exactly the vocab
axis, so PSUM [125, 256] -> SBUF -> strided DMA writes the full 32000-wide
row for each block, zeros included (no scatter needed; duplicate ids are
handled by the matmul accumulation).
"""

import numpy as np
from contextlib import ExitStack

import concourse.bacc as bacc
import concourse.bass as bass
import concourse.mybir as mybir
import concourse.tile as tile
from concourse.bass_utils import run_bass_kernel_spmd

P = 128          # SBUF partitions == positions per block
BLOCK = 128
NB = 128         # blocks per batch (L // BLOCK)
L = NB * BLOCK   # 16384
H = 2
VOCAB = 32000
MHI = 125        # vocab = MHI * NLO
NLO = 256
GROUP = 16       # blocks per output DMA
N_CORES = 8

f32 = mybir.dt.float32
bf16 = mybir.dt.bfloat16
i32 = mybir.dt.int32
i16 = mybir.dt.int16


def build_nc(copy_split=0, iters=1, hw_loop=0):
    """One core's program: tok [128, 256] int32 -> hist [128, 32000] f32.

    tok layout: tok[pos, 2*b + h] = tokens[b*128 + pos, h] (host pre-arranged
    so the DMA in is fully contiguous).
    copy_split: 0 = all PSUM->SBUF copies on the scalar (ACT) engine;
    k>0 = every k-th copy goes to the vector engine instead.
    iters: emit the whole body this many times (benchmarking only).
    hw_loop: if > 0, wrap the body in a tc.For_i hardware loop with this
    trip count (benchmarking only; iters must be 1).
    """
    nc = bacc.Bacc("TRN2", target_bir_lowering=False, debug=False)
    tok_in = nc.dram_tensor("tok", [P, NB * H], i32, kind="ExternalInput")
    hist_out = nc.dram_tensor("hist", [NB, VOCAB], f32, kind="ExternalOutput")

    # Blocks per group: full-size groups for DMA efficiency, tapered at the
    # end so the post-DVE tail (matmul -> copy -> DMA of the last group) is
    # short.
    group_sizes = [GROUP] * (NB // GROUP - 1) + [GROUP // 2, GROUP // 4,
                                                GROUP // 8, GROUP // 8]
    assert sum(group_sizes) == NB
    C = GROUP * H  # max token columns per group

    with ExitStack() as ctx:
        tc = ctx.enter_context(tile.TileContext(nc))
        const = ctx.enter_context(tc.tile_pool(name="const", bufs=1))
        work = ctx.enter_context(tc.tile_pool(name="work", bufs=1))
        oh_pool = ctx.enter_context(tc.tile_pool(name="onehot", bufs=3))
        psum = ctx.enter_context(tc.tile_pool(name="psum", bufs=8, space="PSUM"))
        out_pool = ctx.enter_context(tc.tile_pool(name="outp", bufs=5))

        # The DVE 2x fast path requires every operand's LAST AP dim to be
        # stride +-1, count >= 2, 2-byte dtype. So the one-hot is_equal runs
        # over 4D [P, bins, C/2, 2] views whose innermost pair is contiguous
        # on both sides: tokens use their natural (c_hi, c_lo) split, and the
        # iota table is replicated x2 along a trailing dim (iota2[p, n, y]=n),
        # broadcast (stride 0) over the middle C/2 dim.
        C2 = C // 2
        iota2 = const.tile([P, NLO, 2], i16)
        nc.gpsimd.iota(iota2[:, :, :], pattern=[[1, NLO], [0, 2]], base=0,
                       channel_multiplier=0)

        if hw_loop:
            assert iters == 1
            ctx.enter_context(tc.For_i(0, hw_loop, 1))

        for _ in range(iters):
            tok = work.tile([P, NB * H], i32, tag="tok")
            nc.sync.dma_start(tok[:], tok_in[:])

            # hi = tok >> 8, lo = tok & 255 (exact), kept in int16 (the
            # is_equal compares int16 vs int16, writing bf16 0/1).
            # On GPSIMD: the vector engine is the critical resource.
            hi32 = work.tile([P, NB * H], i32, tag="hi32")
            lo32 = work.tile([P, NB * H], i32, tag="lo32")
            nc.vector.tensor_scalar(
                hi32[:], tok[:], 8, None, mybir.AluOpType.logical_shift_right
            )
            nc.vector.tensor_scalar(
                lo32[:], tok[:], 255, None, mybir.AluOpType.bitwise_and
            )
            hi_bf = work.tile([P, NB * H], i16, tag="hi_bf")
            lo_bf = work.tile([P, NB * H], i16, tag="lo_bf")
            nc.gpsimd.tensor_copy(hi_bf[:], hi32[:])
            nc.gpsimd.tensor_copy(lo_bf[:], lo32[:])

            b0 = 0
            for gs in group_sizes:
                cs = b0 * H
                gc = gs * H    # token columns in this group
                gc2 = gc // 2
                A = oh_pool.tile([P, MHI, C2, 2], bf16, tag="A")
                Bt = oh_pool.tile([P, NLO, C2, 2], bf16, tag="B")
                hi_g = hi_bf[:, cs:cs + gc].rearrange(
                    "p (x y) -> p x y", y=2)[:, None, :, :]
                lo_g = lo_bf[:, cs:cs + gc].rearrange(
                    "p (x y) -> p x y", y=2)[:, None, :, :]
                nc.vector.tensor_tensor(
                    A[:, :, :gc2, :],
                    hi_g.to_broadcast([P, MHI, gc2, 2]),
                    iota2[:, :MHI, None, :].to_broadcast([P, MHI, gc2, 2]),
                    mybir.AluOpType.is_equal,
                )
                nc.vector.tensor_tensor(
                    Bt[:, :, :gc2, :],
                    lo_g.to_broadcast([P, NLO, gc2, 2]),
                    iota2[:, :, None, :].to_broadcast([P, NLO, gc2, 2]),
                    mybir.AluOpType.is_equal,
                )
                og = out_pool.tile([P, GROUP, NLO], f32, tag="og")
                for t in range(gs // 2):
                    # Two blocks share one PSUM bank -> half the copy count.
                    pt = psum.tile([MHI, 2, NLO], f32, tag="pt")
                    for u in range(2):
                        j = 2 * t + u
                        for h in range(2):
                            c = 2 * j + h
                            nc.tensor.matmul(
                                pt[:, u, :],
                                A[:, :, c // 2, c % 2],
                                Bt[:, :, c // 2, c % 2],
                                start=(h == 0), stop=(h == 1),
                            )
                    if copy_split and (t % copy_split == copy_split - 1):
                        nc.vector.tensor_copy(
                            og[:MHI, 2 * t:2 * t + 2, :], pt[:, :, :]
                        )
                    else:
                        nc.scalar.copy(og[:MHI, 2 * t:2 * t + 2, :], pt[:, :, :])
                nc.sync.dma_start(
                    hist_out[b0:b0 + gs, :].rearrange(
                        "b (m l) -> m b l", m=MHI
                    ),
                    og[:MHI, :gs, :],
                )
                b0 += gs
    nc.compile()
    return nc


_CACHE = {}


def _get_nc():
    if "nc" not in _CACHE:
        _CACHE["nc"] = build_nc()
    return _CACHE["nc"]


def shard_tokens(tokens):
    """Per-core input maps: batch i -> core i, rearranged to [pos, (block, h)]."""
    B_ = tokens.shape[0]
    maps = []
    for i in range(B_):
        tok_sb = np.ascontiguousarray(
            tokens[i]
            .reshape(NB, BLOCK, H)
            .transpose(1, 0, 2)
            .reshape(BLOCK, NB * H)
            .astype(np.int32)
        )
        maps.append({"tok": tok_sb})
    return maps


def kernel(tokens, cat_embed_f, W_num):
    tokens = np.asarray(tokens)
    B_, L_, H_ = tokens.shape
    assert (L_, H_) == (L, H) and B_ == N_CORES

    # Passthrough outputs (pure reshuffles of the input)
    cat_ids = np.ascontiguousarray(
        tokens.reshape(B_, NB, BLOCK, H_)[:, :, 0, :].reshape(B_, NB * H_)
    ).astype(np.int32)
    new_tokens = np.concatenate(
        [cat_ids, tokens.reshape(B_, L_ * H_)], axis=1
    ).astype(np.int32)

    nc = _get_nc()
    res = run_bass_kernel_spmd(nc, shard_tokens(tokens), list(range(N_CORES)))
    hist = np.stack(
        [res.results[i]["hist"] for i in range(B_)], axis=0
    ).astype(np.float32)
    return new_tokens, cat_ids, hist
